# revision 16
# baseline (speedup 1.0000x reference)
"""Trainium2 Bass kernel for nn_ButterflyFactorNewMlp.

Computes: attn = einsum('ds,td->st', w1, w2) * sparse_mask
          out  = gelu(einsum('bds,st->bdt', x, attn) + b2)   (exact erf gelu)

Key structural fact (verified against the reference mask): mask[s,t] != 0
iff  s//81 == t//81  and  (s%27)//3 == (t%27)//3.  Writing
s = 81A + 27B + 3C + D, the condition is A_s==A_t and C_s==C_t — so under
the permutation s -> (A, C, B, D) the masked attn becomes block-diagonal
with 81 DENSE 9x9 blocks (6561 nonzeros total).

Sharding (chosen over the data-parallel hint): shard the OUTPUT feature
axis t across the 8 cores.  Core c owns 91-92 consecutive permuted
t-columns; those only read the 99 permuted s-rows of the 11 groups they
straddle.  Every core:
  - receives x^T pre-permuted/transposed on host: [99, 49152] fp16,
  - computes its own [99, 96] attn patch from w1/w2 column slices
    (23 accumulating fp16 matmuls over the 2916 hidden dims + DVE
    mask-multiply),
  - streams all 49152 tokens through ONE stationary-weight matmul per
    512-token chunk (the attn patch stays PE-stationary), erf-gelu out
    of PSUM in 2048-wide ACTIVATEs with b2 applied via the per-partition
    bias port, fp16 out^T stores.

Per-core traffic ~21.5MB (x^T 9.6 + out^T 9.4 + weights 2.3) with no
collectives.  Host does the cheap permute/transpose on both ends.

Trace-driven engine/DMA shaping (measured on HW):
  - A transfer's packets round-robin over C SDMA engines where C = the
    largest divisor of its partition count <= 16.  Bulk transfers are
    issued at 96/128 partitions (C=16); the 3 leftover x rows go as a
    tiny SWDGE DMA on the otherwise-idle gpsimd queue.
  - Descriptors must be FAT: 8KB-per-partition single descriptors run at
    ~26GB/s/engine; splitting them into 1KB pieces drops to ~18.
  - A HWDGE dma_start costs ~760ns on the ISSUING engine, and only
    SP/Activation can issue HWDGE.  ACTIVATEs (~2us each) already load
    the scalar queue, so out-stores ride the sync queue, interleaved
    behind a 5-macro x-in lead so store issues never stall the in-stream.

Precision: fp16 inputs/weights, fp32 PSUM accumulation, erf-gelu LUT on
the fp32 accumulator, fp16 stores — end-to-end ~7e-4 relative error.
"""

import sys

if "/opt/trn_rl_repo" not in sys.path:
    sys.path.insert(0, "/opt/trn_rl_repo")

import numpy as np

import concourse.bacc as bacc
import concourse.mybir as mybir
import concourse.tile as tile
from concourse.bass import ds
from concourse.bass_utils import run_bass_kernel_spmd

F32 = mybir.dt.float32
F16 = mybir.dt.float16
GELU = mybir.ActivationFunctionType.Gelu

N_CORES = 8
B, D, S = 64, 768, 729          # batch, channels, features (729 = in = out)
H = 2916                        # hidden dim of the weight contraction
HP = 2944                       # hidden padded to 23*128
N_KD = HP // 128                # 23 contraction chunks for the attn matmuls
M = B * D                       # 49152 tokens (shared by every core)
KR = 99                         # s-rows per core (11 groups x 9)
KSPL = 96                       # x-in partition split: 96 (C=16) + 3 (gpsimd)
W = 96                          # t-columns per core, padded to C=16
CHUNK = 512                     # tokens per matmul (one PSUM bank)
MACRO = 4096                    # tokens per DMA transfer
N_MACRO = M // MACRO            # 12
GRP = 2048                      # tokens per PSUM tile / ACTIVATE call
XLEAD = 5                       # x-in macros issued ahead of out-store issues

# permuted t-column boundaries per core (92 cols for core 3, 91 otherwise);
# every core's columns straddle exactly 11 of the 81 groups -> 99 s-rows
BOUNDS = [0, 91, 182, 273, 365, 456, 547, 638, 729]

_COMPILED = None
LAST = None  # BassKernelResults of the most recent kernel() call (for test.py)


def _perm():
    s = np.arange(S)
    key = (s // 81) * 81 + ((s % 27) // 3) * 9 + ((s % 81) // 27) * 3 + (s % 3)
    return np.argsort(key, kind="stable")


def _build():
    nc = bacc.Bacc("TRN2", target_bir_lowering=False, debug=False)

    x_d = nc.dram_tensor("x", [KR, M], F16, kind="ExternalInput")
    # weights pre-packed on host into the SBUF-resident layout
    w1_d = nc.dram_tensor("w1s", [128, N_KD, KR], F16, kind="ExternalInput")
    w2_d = nc.dram_tensor("w2ts", [128, N_KD, W], F16, kind="ExternalInput")
    mask_d = nc.dram_tensor("maskc", [KR, W], F16, kind="ExternalInput")
    b2_d = nc.dram_tensor("b2c", [W, 1], F32, kind="ExternalInput")
    out_d = nc.dram_tensor("out", [W, M], F16, kind="ExternalOutput")

    with tile.TileContext(nc) as tc:
        with (
            tc.tile_pool(name="const", bufs=1) as cpool,
            tc.tile_pool(name="xin", bufs=XLEAD + 1) as xpool,
            tc.tile_pool(name="oout", bufs=3) as opool,
            tc.tile_pool(name="ps", bufs=2, space="PSUM") as pspool,
        ):
            # ------- stage 1: this core's [99, 96] attn patch -------
            mask_sb = cpool.tile([128, W], F16)
            nc.scalar.dma_start(mask_sb[0:KR, :], mask_d[:])
            b2_sb = cpool.tile([128, 1], F32)
            nc.scalar.dma_start(b2_sb[0:W, :], b2_d[:])
            w1_sb = cpool.tile([128, N_KD, KR], F16)
            w2_sb = cpool.tile([128, N_KD, W], F16)
            nc.scalar.dma_start(w1_sb[:], w1_d[:])
            nc.scalar.dma_start(w2_sb[:], w2_d[:])

            attn_sb = cpool.tile([128, W], F16)
            ps1 = pspool.tile([128, 4, CHUNK], F32, tag="ps", name="ps1")
            for kd in range(N_KD):
                nc.tensor.matmul(
                    ps1[0:KR, 0, 0:W],
                    w1_sb[:, kd, :],
                    w2_sb[:, kd, :],
                    start=(kd == 0),
                    stop=(kd == N_KD - 1),
                )
            nc.vector.tensor_tensor(
                attn_sb[0:KR, :], ps1[0:KR, 0, 0:W], mask_sb[0:KR, :],
                mybir.AluOpType.mult,
            )

            # ------- stage 2: stream all tokens through the patch -------
            x_tiles = []

            def issue_xin(mi):
                x_sb = xpool.tile([128, MACRO], F16, tag="x")
                x_tiles.append(x_sb)
                win = ds(mi * MACRO, MACRO)
                nc.sync.dma_start(x_sb[0:KSPL, :], x_d[0:KSPL, win])
                nc.gpsimd.dma_start(x_sb[KSPL:KR, :], x_d[KSPL:KR, win])

            for mi in range(min(XLEAD, N_MACRO)):
                issue_xin(mi)

            for mi in range(N_MACRO):
                x_sb = x_tiles[mi]
                o_sb = opool.tile([128, MACRO], F16, tag="o")
                for half in range(MACRO // GRP):
                    ps = pspool.tile([128, 4, CHUNK], F32, tag="ps", name="ps2")
                    for q in range(GRP // CHUNK):
                        si = half * (GRP // CHUNK) + q
                        nc.tensor.matmul(
                            ps[0:W, q, :],
                            attn_sb[0:KR, :],
                            x_sb[0:KR, ds(si * CHUNK, CHUNK)],
                            start=True,
                            stop=True,
                        )
                    # one wide ACTIVATE per 2048 tokens (4 PSUM banks);
                    # b2 rides the per-partition bias port: gelu(x + b2)
                    nc.scalar.activation(
                        o_sb[0:W, ds(half * GRP, GRP)],
                        ps[0:W, :, :],
                        GELU,
                        bias=b2_sb[0:W, :],
                    )
                nc.sync.dma_start(
                    out_d[:, ds(mi * MACRO, MACRO)], o_sb[0:W, :]
                )
                if mi + XLEAD < N_MACRO:
                    issue_xin(mi + XLEAD)

    nc.compile()
    return nc


def _pack_weights(wc):
    """[HP, F] -> [128, N_KD, F]: partition-major SBUF-resident layout."""
    hp, f = wc.shape
    return np.ascontiguousarray(wc.reshape(N_KD, 128, f).transpose(1, 0, 2))


def kernel(x, w1, w2, b2, sparse_mask):
    global _COMPILED, LAST
    if _COMPILED is None:
        _COMPILED = _build()
    nc = _COMPILED

    x = np.asarray(x, dtype=np.float32)
    w1 = np.asarray(w1, dtype=np.float32)
    w2 = np.asarray(w2, dtype=np.float32)
    b2 = np.asarray(b2, dtype=np.float32)
    mask = np.asarray(sparse_mask, dtype=np.float32)

    perm = _perm()
    xh = x.reshape(M, S).astype(np.float16)
    xTp = xh.T[perm]                       # [729, 49152] permuted x^T
    w1p = w1[:, perm]
    w2p = w2[perm, :]
    maskp = mask[np.ix_(perm, perm)]
    b2p = b2[perm]

    in_maps = []
    for c in range(N_CORES):
        t0, t1 = BOUNDS[c], BOUNDS[c + 1]
        wid = t1 - t0
        s0 = 9 * (t0 // 9)
        xc = np.ascontiguousarray(xTp[s0 : s0 + KR])
        w1c = np.zeros((HP, KR), np.float16)
        w1c[:H] = w1p[:, s0 : s0 + KR]
        w2c = np.zeros((HP, W), np.float16)
        w2c[:H, 0:wid] = w2p[t0:t1].T
        mc = np.zeros((KR, W), np.float16)
        mc[:, 0:wid] = maskp[s0 : s0 + KR, t0:t1]
        bc = np.zeros((W, 1), np.float32)
        bc[0:wid, 0] = b2p[t0:t1]
        in_maps.append(
            {
                "x": xc,
                "w1s": _pack_weights(w1c),
                "w2ts": _pack_weights(w2c),
                "maskc": mc,
                "b2c": bc,
            }
        )

    LAST = run_bass_kernel_spmd(nc, in_maps, list(range(N_CORES)))

    outp = np.empty((S, M), np.float16)
    for c in range(N_CORES):
        t0, t1 = BOUNDS[c], BOUNDS[c + 1]
        outp[t0:t1] = LAST.results[c]["out"][0 : t1 - t0]
    final = np.empty((M, S), np.float32)
    final[:, perm] = outp.T
    return final.reshape(B, D, S)


# revision 23
# speedup vs baseline: 1.0364x; 1.0364x over previous
"""Trainium2 Bass kernel for nn_ButterflyFactorNewMlp.

Computes: attn = einsum('ds,td->st', w1, w2) * sparse_mask
          out  = gelu(einsum('bds,st->bdt', x, attn) + b2)   (exact erf gelu)

Key structural fact (verified against the reference mask): mask[s,t] != 0
iff  s//81 == t//81  and  (s%27)//3 == (t%27)//3.  Writing
s = 81A + 27B + 3C + D, the condition is A_s==A_t and C_s==C_t — so under
the permutation s -> (A, C, B, D) the masked attn becomes block-diagonal
with 81 DENSE 9x9 blocks (6561 nonzeros total).

Sharding (chosen over the data-parallel hint): shard the OUTPUT feature
axis t across the 8 cores.  Core c owns 91-92 consecutive permuted
t-columns; those only read the 99 permuted s-rows of the 11 groups they
straddle.  Every core:
  - receives x^T pre-permuted/transposed on host: [99, 49152] fp16,
  - computes its own [99, 96] attn patch from w1/w2 column slices
    (23 accumulating fp16 matmuls over the 2916 hidden dims + DVE
    mask-multiply),
  - streams all 49152 tokens through ONE stationary-weight matmul per
    512-token chunk (the attn patch stays PE-stationary), erf-gelu out
    of PSUM in 2048-wide ACTIVATEs with b2 applied via the per-partition
    bias port, fp16 out^T stores.

Per-core traffic ~21.5MB (x^T 9.6 + out^T 9.4 + weights 2.3) with no
collectives.  Host does the cheap permute/transpose on both ends.

Trace-driven engine/DMA shaping (measured on HW):
  - A transfer's packets round-robin over C SDMA engines where C = the
    largest divisor of its partition count <= 16.  Bulk transfers are
    issued at 96/128 partitions (C=16); the 3 leftover x rows go as a
    tiny SWDGE DMA on the otherwise-idle gpsimd queue.
  - A single DGE ring dispatches only ~205GB/s no matter how many
    engines it fans to, so the three big streams ride three different
    rings: x-in on the sync HWDGE ring, out-stores on a SWDGE queue, and
    weights on the scalar HWDGE ring (issued BEFORE the x flood so
    stage 1 isn't starved; split in halves so the stage-1 matmuls
    pipeline with them).
  - Per-partition packets carry ~150ns of fixed cost on top of wire
    time, so x-in moves in 16KB-per-partition transfers (MACRO=8192).
  - A HWDGE dma_start costs ~760ns on the ISSUING engine and only
    SP/Activation can issue HWDGE; ACTIVATEs (~2us each) already load
    the scalar queue, which is why stores go SWDGE (gpsimd is free).

Precision: fp16 inputs/weights, fp32 PSUM accumulation, erf-gelu LUT on
the fp32 accumulator, fp16 stores — end-to-end ~7e-4 relative error.
"""

import sys

if "/opt/trn_rl_repo" not in sys.path:
    sys.path.insert(0, "/opt/trn_rl_repo")

import numpy as np

import concourse.bacc as bacc
import concourse.mybir as mybir
import concourse.tile as tile
from concourse.bass import ds
from concourse.bass_utils import run_bass_kernel_spmd

F32 = mybir.dt.float32
F16 = mybir.dt.float16
GELU = mybir.ActivationFunctionType.Gelu

N_CORES = 8
B, D, S = 64, 768, 729          # batch, channels, features (729 = in = out)
H = 2916                        # hidden dim of the weight contraction
HP = 2944                       # hidden padded to 23*128
N_KD = HP // 128                # 23 contraction chunks for the attn matmuls
M = B * D                       # 49152 tokens (shared by every core)
KR = 99                         # s-rows per core (11 groups x 9)
KSPL = 96                       # x-in partition split: 96 (C=16) + 3 (gpsimd)
W = 96                          # t-columns per core, padded to C=16
CHUNK = 512                     # tokens per matmul (one PSUM bank)
MACRO = 8192                    # tokens per x-in DMA (16KB per partition)
N_MACRO = M // MACRO            # 6
STORE = 4096                    # tokens per out-store DMA
GRP = 2048                      # tokens per PSUM tile / ACTIVATE call
XLEAD = 3                       # x-in macros prefetched ahead of compute

# permuted t-column boundaries per core (92 cols for core 3, 91 otherwise);
# every core's columns straddle exactly 11 of the 81 groups -> 99 s-rows
BOUNDS = [0, 91, 182, 273, 365, 456, 547, 638, 729]

_COMPILED = None
LAST = None  # BassKernelResults of the most recent kernel() call (for test.py)


def _perm():
    s = np.arange(S)
    key = (s // 81) * 81 + ((s % 27) // 3) * 9 + ((s % 81) // 27) * 3 + (s % 3)
    return np.argsort(key, kind="stable")


def _build():
    nc = bacc.Bacc("TRN2", target_bir_lowering=False, debug=False)

    x_d = nc.dram_tensor("x", [KR, M], F16, kind="ExternalInput")
    # weights pre-packed on host into the SBUF-resident layout
    w1_d = nc.dram_tensor("w1s", [128, N_KD, KR], F16, kind="ExternalInput")
    w2_d = nc.dram_tensor("w2ts", [128, N_KD, W], F16, kind="ExternalInput")
    mask_d = nc.dram_tensor("maskc", [KR, W], F16, kind="ExternalInput")
    b2_d = nc.dram_tensor("b2c", [W, 1], F32, kind="ExternalInput")
    out_d = nc.dram_tensor("out", [W, M], F16, kind="ExternalOutput")

    with tile.TileContext(nc) as tc:
        with (
            tc.tile_pool(name="const", bufs=1) as cpool,
            tc.tile_pool(name="xin", bufs=XLEAD + 1) as xpool,
            tc.tile_pool(name="oout", bufs=3) as opool,
            tc.tile_pool(name="ps", bufs=2, space="PSUM") as pspool,
        ):
            # ------- stage 1: this core's [99, 96] attn patch -------
            # weights lead both HWDGE rings (w1 halves on scalar, w2 on
            # sync ahead of the x flood) so stage 1 is ready by ~10us;
            # halves let the stage-1 matmuls pipeline with the DMAs
            KD_A = 12
            mask_sb = cpool.tile([128, W], F16)
            nc.scalar.dma_start(mask_sb[0:KR, :], mask_d[:])
            b2_sb = cpool.tile([128, 1], F32)
            nc.scalar.dma_start(b2_sb[0:W, :], b2_d[:])
            w1_sb = cpool.tile([128, N_KD, KR], F16)
            w2_sb = cpool.tile([128, N_KD, W], F16)
            nc.scalar.dma_start(w1_sb[:, 0:KD_A, :], w1_d[:, 0:KD_A, :])
            nc.sync.dma_start(w2_sb[:, 0:KD_A, :], w2_d[:, 0:KD_A, :])
            nc.scalar.dma_start(w1_sb[:, KD_A:N_KD, :], w1_d[:, KD_A:N_KD, :])
            nc.sync.dma_start(w2_sb[:, KD_A:N_KD, :], w2_d[:, KD_A:N_KD, :])

            attn_sb = cpool.tile([128, W], F16)
            ps1 = pspool.tile([128, 4, CHUNK], F32, tag="ps", name="ps1")
            for kd in range(N_KD):
                nc.tensor.matmul(
                    ps1[0:KR, 0, 0:W],
                    w1_sb[:, kd, :],
                    w2_sb[:, kd, :],
                    start=(kd == 0),
                    stop=(kd == N_KD - 1),
                )
            nc.vector.tensor_tensor(
                attn_sb[0:KR, :], ps1[0:KR, 0, 0:W], mask_sb[0:KR, :],
                mybir.AluOpType.mult,
            )

            # ------- stage 2: stream all tokens through the patch -------
            x_tiles = []

            def issue_xin(mi):
                x_sb = xpool.tile([128, MACRO], F16, tag="x")
                x_tiles.append(x_sb)
                win = ds(mi * MACRO, MACRO)
                nc.sync.dma_start(x_sb[0:KSPL, :], x_d[0:KSPL, win])
                nc.gpsimd.dma_start(x_sb[KSPL:KR, :], x_d[KSPL:KR, win])

            for mi in range(min(XLEAD, N_MACRO)):
                issue_xin(mi)

            for mi in range(N_MACRO):
                x_sb = x_tiles[mi]
                o_sb = opool.tile([128, MACRO], F16, tag="o")
                for half in range(MACRO // GRP):
                    ps = pspool.tile([128, 4, CHUNK], F32, tag="ps", name="ps2")
                    for q in range(GRP // CHUNK):
                        si = half * (GRP // CHUNK) + q
                        nc.tensor.matmul(
                            ps[0:W, q, :],
                            attn_sb[0:KR, :],
                            x_sb[0:KR, ds(si * CHUNK, CHUNK)],
                            start=True,
                            stop=True,
                        )
                    # one wide ACTIVATE per 2048 tokens (4 PSUM banks);
                    # b2 rides the per-partition bias port: gelu(x + b2)
                    nc.scalar.activation(
                        o_sb[0:W, ds(half * GRP, GRP)],
                        ps[0:W, :, :],
                        GELU,
                        bias=b2_sb[0:W, :],
                    )
                    # store every 4096 tokens on the second SWDGE queue
                    if half % (STORE // GRP) == (STORE // GRP) - 1:
                        h0 = half * GRP - (STORE - GRP)
                        nc.gpsimd.dma_start(
                            out_d[:, ds(mi * MACRO + h0, STORE)],
                            o_sb[0:W, ds(h0, STORE)],
                        )
                if mi + XLEAD < N_MACRO:
                    issue_xin(mi + XLEAD)

    nc.compile()
    return nc


def _pack_weights(wc):
    """[HP, F] -> [128, N_KD, F]: partition-major SBUF-resident layout."""
    hp, f = wc.shape
    return np.ascontiguousarray(wc.reshape(N_KD, 128, f).transpose(1, 0, 2))


def kernel(x, w1, w2, b2, sparse_mask):
    global _COMPILED, LAST
    if _COMPILED is None:
        _COMPILED = _build()
    nc = _COMPILED

    x = np.asarray(x, dtype=np.float32)
    w1 = np.asarray(w1, dtype=np.float32)
    w2 = np.asarray(w2, dtype=np.float32)
    b2 = np.asarray(b2, dtype=np.float32)
    mask = np.asarray(sparse_mask, dtype=np.float32)

    perm = _perm()
    xh = x.reshape(M, S).astype(np.float16)
    xTp = xh.T[perm]                       # [729, 49152] permuted x^T
    w1p = w1[:, perm]
    w2p = w2[perm, :]
    maskp = mask[np.ix_(perm, perm)]
    b2p = b2[perm]

    in_maps = []
    for c in range(N_CORES):
        t0, t1 = BOUNDS[c], BOUNDS[c + 1]
        wid = t1 - t0
        s0 = 9 * (t0 // 9)
        xc = np.ascontiguousarray(xTp[s0 : s0 + KR])
        w1c = np.zeros((HP, KR), np.float16)
        w1c[:H] = w1p[:, s0 : s0 + KR]
        w2c = np.zeros((HP, W), np.float16)
        w2c[:H, 0:wid] = w2p[t0:t1].T
        mc = np.zeros((KR, W), np.float16)
        mc[:, 0:wid] = maskp[s0 : s0 + KR, t0:t1]
        bc = np.zeros((W, 1), np.float32)
        bc[0:wid, 0] = b2p[t0:t1]
        in_maps.append(
            {
                "x": xc,
                "w1s": _pack_weights(w1c),
                "w2ts": _pack_weights(w2c),
                "maskc": mc,
                "b2c": bc,
            }
        )

    LAST = run_bass_kernel_spmd(nc, in_maps, list(range(N_CORES)))

    outp = np.empty((S, M), np.float16)
    for c in range(N_CORES):
        t0, t1 = BOUNDS[c], BOUNDS[c + 1]
        outp[t0:t1] = LAST.results[c]["out"][0 : t1 - t0]
    final = np.empty((M, S), np.float32)
    final[:, perm] = outp.T
    return final.reshape(B, D, S)


# revision 27
# speedup vs baseline: 1.2374x; 1.1939x over previous
"""Trainium2 Bass kernel for nn_ButterflyFactorNewMlp.

Computes: attn = einsum('ds,td->st', w1, w2) * sparse_mask
          out  = gelu(einsum('bds,st->bdt', x, attn) + b2)   (exact erf gelu)

Key structural fact (verified against the reference mask): mask[s,t] != 0
iff  s//81 == t//81  and  (s%27)//3 == (t%27)//3.  Writing
s = 81A + 27B + 3C + D, the condition is A_s==A_t and C_s==C_t — so under
the permutation s -> (A, C, B, D) the masked attn becomes block-diagonal
with 81 DENSE 9x9 blocks (6561 nonzeros total).

Sharding (chosen over the data-parallel hint): shard the OUTPUT feature
axis t across the 8 cores.  Core c owns 91-92 consecutive permuted
t-columns; those only read the 99 permuted s-rows of the 11 groups they
straddle.  Every core:
  - receives x^T pre-permuted/transposed on host: [99, 49152] fp16,
  - computes its own [99, 96] attn patch from w1/w2 column slices
    (23 accumulating fp16 matmuls over the 2916 hidden dims + DVE
    mask-multiply),
  - streams all 49152 tokens through ONE stationary-weight matmul per
    512-token chunk (the attn patch stays PE-stationary), erf-gelu out
    of PSUM in 2048-wide ACTIVATEs with b2 applied via the per-partition
    bias port, fp16 out^T stores.

Per-core traffic ~21.5MB (x^T 9.6 + out^T 9.4 + weights 2.3) with no
collectives.  Host does the cheap permute/transpose on both ends.

Trace-driven engine/DMA shaping (measured on HW):
  - A transfer's packets round-robin over C SDMA engines where C = the
    largest divisor of its partition count <= 16.  Bulk transfers are
    issued at 96/128 partitions (C=16); the 3 leftover x rows go as a
    tiny SWDGE DMA on the otherwise-idle gpsimd queue.
  - A single DGE ring dispatches only ~205GB/s no matter how many
    engines it fans to, so the three big streams ride three different
    rings: x-in on the sync HWDGE ring, out-stores on a SWDGE queue, and
    weights on the scalar HWDGE ring (issued BEFORE the x flood so
    stage 1 isn't starved; split in halves so the stage-1 matmuls
    pipeline with them).
  - Per-partition packets carry ~150ns of fixed cost on top of wire
    time, so x-in moves in 16KB-per-partition transfers (MACRO=8192).
  - A HWDGE dma_start costs ~760ns on the ISSUING engine and only
    SP/Activation can issue HWDGE; ACTIVATEs (~2us each) already load
    the scalar queue, which is why stores go SWDGE (gpsimd is free).

Precision: fp16 inputs/weights, fp32 PSUM accumulation, erf-gelu LUT on
the fp32 accumulator, fp16 stores — end-to-end ~7e-4 relative error.
"""

import sys

if "/opt/trn_rl_repo" not in sys.path:
    sys.path.insert(0, "/opt/trn_rl_repo")

import numpy as np

import concourse.bacc as bacc
import concourse.mybir as mybir
import concourse.tile as tile
from concourse.bass import ds
from concourse.bass_utils import run_bass_kernel_spmd

F32 = mybir.dt.float32
F16 = mybir.dt.float16
GELU = mybir.ActivationFunctionType.Gelu

N_CORES = 8
B, D, S = 64, 768, 729          # batch, channels, features (729 = in = out)
H = 2916                        # hidden dim of the weight contraction
HP = 2944                       # hidden padded to 23*128
N_KD = HP // 128                # 23 contraction chunks for the attn matmuls
M = B * D                       # 49152 tokens (shared by every core)
KR = 99                         # s-rows per core (11 groups x 9)
KSPL = 96                       # x-in partition split: 96 (C=16) + 3 (gpsimd)
W = 96                          # t-columns per core, padded to C=16
CHUNK = 512                     # tokens per matmul (one PSUM bank)
MACRO = 4096                    # tokens per x-in DMA (8KB per partition)
N_MACRO = M // MACRO            # 12
STORE = 2048                    # tokens per out-store DMA
GRP = 2048                      # tokens per PSUM tile / ACTIVATE call
XLEAD = 4                       # x-in macros prefetched ahead of compute

# permuted t-column boundaries per core (92 cols for core 3, 91 otherwise);
# every core's columns straddle exactly 11 of the 81 groups -> 99 s-rows
BOUNDS = [0, 91, 182, 273, 365, 456, 547, 638, 729]

_COMPILED = None
LAST = None  # BassKernelResults of the most recent kernel() call (for test.py)


def _perm():
    s = np.arange(S)
    key = (s // 81) * 81 + ((s % 27) // 3) * 9 + ((s % 81) // 27) * 3 + (s % 3)
    return np.argsort(key, kind="stable")


def _build():
    nc = bacc.Bacc("TRN2", target_bir_lowering=False, debug=False)

    x_d = nc.dram_tensor("x", [KR, M], F16, kind="ExternalInput")
    # weights pre-packed on host into the flat SBUF-resident layout; flat
    # 2D APs keep each DMA one fat contiguous descriptor per partition
    w1_d = nc.dram_tensor("w1s", [128, N_KD * KR], F16, kind="ExternalInput")
    w2_d = nc.dram_tensor("w2ts", [128, N_KD * W], F16, kind="ExternalInput")
    mask_d = nc.dram_tensor("maskc", [KR, W], F16, kind="ExternalInput")
    b2_d = nc.dram_tensor("b2c", [W, 1], F32, kind="ExternalInput")
    out_d = nc.dram_tensor("out", [W, M], F16, kind="ExternalOutput")

    with tile.TileContext(nc) as tc:
        with (
            tc.tile_pool(name="const", bufs=1) as cpool,
            tc.tile_pool(name="xin", bufs=XLEAD + 1) as xpool,
            tc.tile_pool(name="oout", bufs=3) as opool,
            tc.tile_pool(name="ps", bufs=2, space="PSUM") as pspool,
        ):
            # ------- stage 1: this core's [99, 96] attn patch -------
            # weights lead both HWDGE rings (w1 halves on scalar, w2 on
            # sync ahead of the x flood) so stage 1 is ready by ~11us;
            # halves let the stage-1 matmuls pipeline with the DMAs
            KD_A = 12
            w1_sb = cpool.tile([128, N_KD * KR], F16)
            w2_sb = cpool.tile([128, N_KD * W], F16)
            nc.scalar.dma_start(
                w1_sb[:, 0 : KD_A * KR], w1_d[:, 0 : KD_A * KR]
            )
            nc.sync.dma_start(w2_sb[:, 0 : KD_A * W], w2_d[:, 0 : KD_A * W])
            nc.scalar.dma_start(
                w1_sb[:, KD_A * KR :], w1_d[:, KD_A * KR :]
            )
            nc.sync.dma_start(w2_sb[:, KD_A * W :], w2_d[:, KD_A * W :])
            b2_sb = cpool.tile([128, 1], F32)
            nc.scalar.dma_start(b2_sb[0:W, :], b2_d[:])
            mask_sb = cpool.tile([128, W], F16)
            nc.scalar.dma_start(mask_sb[0:KR, :], mask_d[:])
            # warm the Gelu ACT table (~2.6us load) before the real stream
            scratch = cpool.tile([128, 1], F16)
            nc.scalar.activation(scratch[0:W, :], b2_sb[0:W, :], GELU)

            attn_sb = cpool.tile([128, W], F16)
            ps1 = pspool.tile([128, 4, CHUNK], F32, tag="ps", name="ps1")
            for kd in range(N_KD):
                nc.tensor.matmul(
                    ps1[0:KR, 0, 0:W],
                    w1_sb[:, ds(kd * KR, KR)],
                    w2_sb[:, ds(kd * W, W)],
                    start=(kd == 0),
                    stop=(kd == N_KD - 1),
                )
            nc.vector.tensor_tensor(
                attn_sb[0:KR, :], ps1[0:KR, 0, 0:W], mask_sb[0:KR, :],
                mybir.AluOpType.mult,
            )

            # ------- stage 2: stream all tokens through the patch -------
            x_tiles = []

            def issue_xin(mi):
                x_sb = xpool.tile([128, MACRO], F16, tag="x")
                x_tiles.append(x_sb)
                win = ds(mi * MACRO, MACRO)
                nc.sync.dma_start(x_sb[0:KSPL, :], x_d[0:KSPL, win])
                nc.gpsimd.dma_start(x_sb[KSPL:KR, :], x_d[KSPL:KR, win])

            for mi in range(min(XLEAD, N_MACRO)):
                issue_xin(mi)

            for mi in range(N_MACRO):
                x_sb = x_tiles[mi]
                o_sb = opool.tile([128, MACRO], F16, tag="o")
                for half in range(MACRO // GRP):
                    ps = pspool.tile([128, 4, CHUNK], F32, tag="ps", name="ps2")
                    for q in range(GRP // CHUNK):
                        si = half * (GRP // CHUNK) + q
                        nc.tensor.matmul(
                            ps[0:W, q, :],
                            attn_sb[0:KR, :],
                            x_sb[0:KR, ds(si * CHUNK, CHUNK)],
                            start=True,
                            stop=True,
                        )
                    # one wide ACTIVATE per 2048 tokens (4 PSUM banks);
                    # b2 rides the per-partition bias port: gelu(x + b2)
                    nc.scalar.activation(
                        o_sb[0:W, ds(half * GRP, GRP)],
                        ps[0:W, :, :],
                        GELU,
                        bias=b2_sb[0:W, :],
                    )
                    # store every 4096 tokens on the second SWDGE queue
                    if half % (STORE // GRP) == (STORE // GRP) - 1:
                        h0 = half * GRP - (STORE - GRP)
                        nc.gpsimd.dma_start(
                            out_d[:, ds(mi * MACRO + h0, STORE)],
                            o_sb[0:W, ds(h0, STORE)],
                        )
                if mi + XLEAD < N_MACRO:
                    issue_xin(mi + XLEAD)

    nc.compile()
    return nc


def _pack_weights(wc):
    """[HP, F] -> [128, N_KD*F]: flat partition-major SBUF-resident layout."""
    hp, f = wc.shape
    return np.ascontiguousarray(
        wc.reshape(N_KD, 128, f).transpose(1, 0, 2).reshape(128, N_KD * f)
    )


def kernel(x, w1, w2, b2, sparse_mask):
    global _COMPILED, LAST
    if _COMPILED is None:
        _COMPILED = _build()
    nc = _COMPILED

    x = np.asarray(x, dtype=np.float32)
    w1 = np.asarray(w1, dtype=np.float32)
    w2 = np.asarray(w2, dtype=np.float32)
    b2 = np.asarray(b2, dtype=np.float32)
    mask = np.asarray(sparse_mask, dtype=np.float32)

    perm = _perm()
    xh = x.reshape(M, S).astype(np.float16)
    xTp = xh.T[perm]                       # [729, 49152] permuted x^T
    w1p = w1[:, perm]
    w2p = w2[perm, :]
    maskp = mask[np.ix_(perm, perm)]
    b2p = b2[perm]

    in_maps = []
    for c in range(N_CORES):
        t0, t1 = BOUNDS[c], BOUNDS[c + 1]
        wid = t1 - t0
        s0 = 9 * (t0 // 9)
        xc = np.ascontiguousarray(xTp[s0 : s0 + KR])
        w1c = np.zeros((HP, KR), np.float16)
        w1c[:H] = w1p[:, s0 : s0 + KR]
        w2c = np.zeros((HP, W), np.float16)
        w2c[:H, 0:wid] = w2p[t0:t1].T
        mc = np.zeros((KR, W), np.float16)
        mc[:, 0:wid] = maskp[s0 : s0 + KR, t0:t1]
        bc = np.zeros((W, 1), np.float32)
        bc[0:wid, 0] = b2p[t0:t1]
        in_maps.append(
            {
                "x": xc,
                "w1s": _pack_weights(w1c),
                "w2ts": _pack_weights(w2c),
                "maskc": mc,
                "b2c": bc,
            }
        )

    LAST = run_bass_kernel_spmd(nc, in_maps, list(range(N_CORES)))

    outp = np.empty((S, M), np.float16)
    for c in range(N_CORES):
        t0, t1 = BOUNDS[c], BOUNDS[c + 1]
        outp[t0:t1] = LAST.results[c]["out"][0 : t1 - t0]
    final = np.empty((M, S), np.float32)
    final[:, perm] = outp.T
    return final.reshape(B, D, S)
